# revision 10
# baseline (speedup 1.0000x reference)
"""Trainium2 Bass kernel for nn_ContextRelation_Module (dense_transformer).

Data-parallel over batch: 8 batches -> 8 NeuronCores, one batch each.

Per-core program (B=1 slice), algebraically restructured vs the module:

  x     [512, 16384]   (C_in, H*W), bf16
  q1'   = relu(Wq1 @ x + bq1/s1)                     [256, T]  (s1 folded into Wq2)
  q2'   = relu(Wq2' @ q1' + bq2/s2)                  [256, T]  (s2 folded into k2)
  sim   = k2'^T @ q2'                                [19, T]
  esim  = exp(sim / 16)                 (ACT, [19,T])
  denom = partition-reduce(esim)        (Pool, [1,T])
  recip = 1/denom                       (DVE custom approx, [1,T])
  recipB= ones19 ^T recip               (PE broadcast matmul, [19,T])
  attT  = esim * recipB  -> bf16        (DVE, [19,T])
  y     = relu(uvT^T @ attT + bu)       [512, T], bf16 out

where uvT = (Wu' @ v)^T [19, 512] is computed once in the preamble --
u-projection folded through the (linear) attention-weighted sum, removing
the [256,T] ctx intermediate entirely.  All BN scales are folded into the
next layer's weights (host- or preamble-side) so every PSUM drain is a
single instruction: relu(psum + bias) on either ACT or DVE.

Big-GEMM operands (x, Wq1, Wq2', k2', uvT, attT) are bf16; accumulation is
fp32 in PSUM; the k/v/uv preamble stays fp32.  x and y move over HBM as
bf16 (y is converted back to fp32 on the host).
"""

import numpy as np

import concourse.bacc as bacc
import concourse.bass as bass
import concourse.bass_isa as bass_isa
import concourse.mybir as mybir
import concourse.tile as tile
from concourse import bass_utils
from concourse.bass import ts

AFT = mybir.ActivationFunctionType
ALU = mybir.AluOpType
F32 = mybir.dt.float32
BF16 = mybir.dt.bfloat16

# problem dims (hardcoded per contract)
B = 8
C = 512            # input/output channels
K = 256            # key_channels
H = 128
W = 128
NCTX = 19          # context tokens
NPIX = H * W       # 16384 pixels per batch
CB = C // 128      # 4 partition blocks of C
KB = K // 128      # 2 partition blocks of K
EPS = 1e-5
SOFTMAX_SCALE = K ** -0.5   # 1/16

# tunables
TN = 512                       # free-dim tile (one PSUM bank of fp32)


def _build(npix=NPIX, repeat=1):
    """Build + compile the per-core Bass module."""
    nt = npix // TN
    nc = bacc.Bacc("TRN2", target_bir_lowering=False, debug=False)

    x_d = nc.dram_tensor("x", [C, npix], BF16, kind="ExternalInput").ap()
    ct_d = nc.dram_tensor("ctxt", [C, NCTX], F32, kind="ExternalInput").ap()
    wq1_d = nc.dram_tensor("wq1", [C, K], BF16, kind="ExternalInput").ap()
    wq2_d = nc.dram_tensor("wq2", [K, K], BF16, kind="ExternalInput").ap()
    wk1_d = nc.dram_tensor("wk1", [C, K], F32, kind="ExternalInput").ap()
    wk2_d = nc.dram_tensor("wk2", [K, K], F32, kind="ExternalInput").ap()
    wv_d = nc.dram_tensor("wv", [C, K], F32, kind="ExternalInput").ap()
    wu_d = nc.dram_tensor("wu", [K, C], F32, kind="ExternalInput").ap()
    # packed per-channel vectors: [128, nblk], channel = blk*128 + p
    sb_names = ["bq1", "bq2", "sk1", "bk1", "sk2", "bk2", "sq2", "sv", "bv", "bu"]
    sb_d = {}
    for n in sb_names:
        nblk = CB if n == "bu" else KB
        sb_d[n] = nc.dram_tensor(n, [128, nblk], F32, kind="ExternalInput").ap()
    y_d = nc.dram_tensor("y", [C, npix], BF16, kind="ExternalOutput").ap()

    x_v = x_d.rearrange("(c p) n -> p c n", p=128)
    y_v = y_d.rearrange("(c p) n -> p c n", p=128)

    with tile.TileContext(nc) as tc, nc.allow_low_precision(reason="bf16 matmul operands"):
        with (
            tc.tile_pool(name="consts", bufs=1) as consts,
            tc.tile_pool(name="xin", bufs=3) as xin,
            tc.tile_pool(name="yout", bufs=3) as yout,
            tc.tile_pool(name="work", bufs=2) as work,
            tc.tile_pool(name="psA", bufs=3, space="PSUM") as psA,      # q1/q2 [128,512]
            tc.tile_pool(name="psS", bufs=2, space="PSUM") as psS,      # sim [19,512]
            tc.tile_pool(name="psY", bufs=3, space="PSUM") as psY,      # y [128,512]
        ):
            # ---- constants ----
            wq1_sb = consts.tile([128, CB, K], BF16, name="wq1_sb")
            nc.sync.dma_start(out=wq1_sb, in_=wq1_d.rearrange("(c p) m -> p c m", p=128))
            wq2_sb = consts.tile([128, KB, K], BF16, name="wq2_sb")
            nc.sync.dma_start(out=wq2_sb, in_=wq2_d.rearrange("(c p) m -> p c m", p=128))
            wk1_sb = consts.tile([128, CB, K], F32, name="wk1_sb")
            nc.sync.dma_start(out=wk1_sb, in_=wk1_d.rearrange("(c p) m -> p c m", p=128))
            wk2_sb = consts.tile([128, KB, K], F32, name="wk2_sb")
            nc.sync.dma_start(out=wk2_sb, in_=wk2_d.rearrange("(c p) m -> p c m", p=128))
            wv_sb = consts.tile([128, CB, K], F32, name="wv_sb")
            nc.sync.dma_start(out=wv_sb, in_=wv_d.rearrange("(c p) m -> p c m", p=128))
            wu_sb = consts.tile([128, KB, C], F32, name="wu_sb")
            nc.sync.dma_start(out=wu_sb, in_=wu_d.rearrange("(c p) m -> p c m", p=128))
            sb = {}
            for n in sb_names:
                nblk = CB if n == "bu" else KB
                t_ = consts.tile([128, nblk], F32, name=f"{n}_sb")
                nc.sync.dma_start(out=t_, in_=sb_d[n])
                sb[n] = t_
            ct_sb = consts.tile([128, CB, NCTX], F32, name="ct_sb")
            nc.sync.dma_start(out=ct_sb, in_=ct_d.rearrange("(c p) m -> p c m", p=128))

            # ---- preamble: k2' (bf16, s_q2-scaled), uvT (bf16) ----
            k1_sb = consts.tile([128, KB, NCTX], F32, name="k1_sb")
            for m in range(KB):
                pf = psA.tile([128, TN], F32, tag="mm", name="pk1")
                p = pf[:, :NCTX]
                for c in range(CB):
                    nc.tensor.matmul(p, wk1_sb[:, c, ts(m, 128)], ct_sb[:, c, :],
                                     start=(c == 0), stop=(c == CB - 1))
                nc.scalar.activation(k1_sb[:, m, :], p, AFT.Relu,
                                     bias=sb["bk1"][:, m:m + 1], scale=sb["sk1"][:, m:m + 1])
            k2_sb = consts.tile([128, KB, NCTX], BF16, name="k2_sb")
            for m in range(KB):
                pf = psA.tile([128, TN], F32, tag="mm", name="pk2")
                p = pf[:, :NCTX]
                for c in range(KB):
                    nc.tensor.matmul(p, wk2_sb[:, c, ts(m, 128)], k1_sb[:, c, :],
                                     start=(c == 0), stop=(c == KB - 1))
                # relu(s*psum+b) then * s_q2 (folded from the q2 BN scale)
                kf = consts.tile([128, NCTX], F32, name=f"k2f{m}")
                nc.scalar.activation(kf, p, AFT.Relu,
                                     bias=sb["bk2"][:, m:m + 1], scale=sb["sk2"][:, m:m + 1])
                nc.vector.tensor_scalar_mul(k2_sb[:, m, :], kf, sb["sq2"][:, m:m + 1])
            v_sb = consts.tile([128, KB, NCTX], F32, name="v_sb")
            for m in range(KB):
                pf = psA.tile([128, TN], F32, tag="mm", name="pv")
                p = pf[:, :NCTX]
                for c in range(CB):
                    nc.tensor.matmul(p, wv_sb[:, c, ts(m, 128)], ct_sb[:, c, :],
                                     start=(c == 0), stop=(c == CB - 1))
                nc.scalar.activation(v_sb[:, m, :], p, AFT.Relu,
                                     bias=sb["bv"][:, m:m + 1], scale=sb["sv"][:, m:m + 1])
            # uvT [19, C] = v^T @ Wu'^T  (s_u already folded into wu host-side)
            puv = psS.tile([NCTX, TN], F32, tag="sim", name="puv")
            for c in range(KB):
                nc.tensor.matmul(puv, v_sb[:, c, :], wu_sb[:, c, :],
                                 start=(c == 0), stop=(c == KB - 1))
            uvT_sb = consts.tile([NCTX, C], BF16, name="uvT_sb")
            nc.scalar.activation(uvT_sb, puv, AFT.Copy)

            # ---- main loop, software-pipelined ----
            state = {}

            def s0a(t):  # x dma + q1 m0
                xt = xin.tile([128, CB, TN], BF16, tag="xt", name="xt")
                nc.sync.dma_start(out=xt, in_=x_v[:, :, ts(t, TN)])
                q1 = work.tile([128, KB, TN], BF16, tag="q1", name="q1")
                p = psA.tile([128, TN], F32, tag="mm", name="pq1a")
                for c in range(CB):
                    nc.tensor.matmul(p, wq1_sb[:, c, ts(0, 128)], xt[:, c, :],
                                     start=(c == 0), stop=(c == CB - 1))
                nc.scalar.activation(q1[:, 0, :], p, AFT.Relu, bias=sb["bq1"][:, 0:1])
                state[t] = {"xt": xt, "q1": q1}

            def s0b(t):  # q1 m1
                st = state[t]
                p = psA.tile([128, TN], F32, tag="mm", name="pq1b")
                for c in range(CB):
                    nc.tensor.matmul(p, wq1_sb[:, c, ts(1, 128)], st["xt"][:, c, :],
                                     start=(c == 0), stop=(c == CB - 1))
                nc.vector.tensor_scalar(st["q1"][:, 1, :], p,
                                        sb["bq1"][:, 1:2], 0.0, ALU.add, ALU.max)

            def s0c(t):  # q2 both blocks
                st = state[t]
                q2 = work.tile([128, KB, TN], BF16, tag="q2", name="q2")
                for m in range(KB):
                    p = psA.tile([128, TN], F32, tag="mm", name="pq2")
                    for c in range(KB):
                        nc.tensor.matmul(p, wq2_sb[:, c, ts(m, 128)], st["q1"][:, c, :],
                                         start=(c == 0), stop=(c == KB - 1))
                    if m == 0:
                        nc.scalar.activation(q2[:, m, :], p, AFT.Relu, bias=sb["bq2"][:, m:m + 1])
                    else:
                        nc.vector.tensor_scalar(q2[:, m, :], p,
                                                sb["bq2"][:, m:m + 1], 0.0, ALU.add, ALU.max)
                st["q2"] = q2

            def s0d(t):  # sim [19, TN]
                st = state[t]
                psim = psS.tile([NCTX, TN], F32, tag="sim", name="psim")
                for c in range(KB):
                    nc.tensor.matmul(psim, k2_sb[:, c, :], st["q2"][:, c, :],
                                     start=(c == 0), stop=(c == KB - 1))
                st["psim"] = psim

            def s1(t):  # exp + all-reduce denom (replicated across the 19 rows)
                st = state[t]
                esim = work.tile([NCTX, TN], F32, tag="esim", name="esim")
                nc.scalar.activation(esim, st["psim"], AFT.Exp, scale=SOFTMAX_SCALE)
                denomB = work.tile([NCTX, TN], F32, tag="denomB", name="denomB")
                nc.gpsimd.partition_all_reduce(denomB, esim, channels=NCTX,
                                               reduce_op=bass_isa.ReduceOp.add)
                st["esim"] = esim
                st["denomB"] = denomB

            def s2(t):  # recip + normalize -> bf16
                st = state[t]
                recipB = work.tile([NCTX, TN], F32, tag="recipB", name="recipB")
                nc.vector.reciprocal_approx_fast(out=recipB, in_=st["denomB"])
                attT = work.tile([NCTX, TN], BF16, tag="attT", name="attT")
                nc.vector.tensor_mul(attT, st["esim"], recipB)
                st["attT"] = attT

            def s3(t, ms):  # y blocks
                st = state[t]
                if "yt" not in st:
                    st["yt"] = yout.tile([128, CB, TN], BF16, tag="yt", name="yt")
                yt = st["yt"]
                for m in ms:
                    p = psY.tile([128, TN], F32, tag="y", name="py")
                    nc.tensor.matmul(p, uvT_sb[:, ts(m, 128)], st["attT"],
                                     start=True, stop=True)
                    if m % 2 == 0:
                        nc.scalar.activation(yt[:, m, :], p, AFT.Relu, bias=sb["bu"][:, m:m + 1])
                    else:
                        nc.vector.tensor_scalar(yt[:, m, :], p,
                                                sb["bu"][:, m:m + 1], 0.0, ALU.add, ALU.max)
                if ms[-1] == CB - 1:
                    nc.gpsimd.dma_start(out=y_v[:, :, ts(t, TN)], in_=yt)
                    state.pop(t)

            for r in range(repeat):
                for t in range(nt + 5):
                    if 4 <= t <= nt + 3:
                        s3(t - 4, [0, 1])
                    if t < nt:
                        s0a(t)
                    if 1 <= t <= nt:
                        s0d(t - 1)
                    if 4 <= t <= nt + 3:
                        s3(t - 4, [2, 3])
                    if t < nt:
                        s0b(t)
                    if 2 <= t <= nt + 1:
                        s1(t - 2)
                    if t < nt:
                        s0c(t)
                    if 3 <= t <= nt + 2:
                        s2(t - 3)

    nc.compile()
    return nc


def _prepare_inputs(inputs, npix=NPIX):
    """Fold BN into weights/biases, transpose, shard over batch."""
    import ml_dtypes
    f = np.float32
    bf = ml_dtypes.bfloat16

    def fold(bn, conv_b):
        g, be, m, v = [np.asarray(a, dtype=np.float64) for a in bn]
        s = g / np.sqrt(v + EPS)
        t = be - m * s
        bias = np.asarray(conv_b, dtype=np.float64) * s + t
        return s, bias

    def pack(vec):  # [C'] -> [128, C'//128], channel = blk*128 + p
        return np.ascontiguousarray(np.asarray(vec, f).reshape(-1, 128).T)

    s1, b1 = fold(inputs["qbn1"], inputs["qb1"])
    s2, b2 = fold(inputs["qbn2"], inputs["qb2"])
    sk1, bk1 = fold(inputs["kbn1"], inputs["kb1"])
    sk2, bk2 = fold(inputs["kbn2"], inputs["kb2"])
    sv, bv = fold(inputs["vbn"], inputs["vb"])
    su, bu = fold(inputs["ubn"], inputs["ub"])

    qW2 = np.asarray(inputs["qW2"], np.float64)
    uW = np.asarray(inputs["uW"], np.float64)

    base = {
        "wq1": np.ascontiguousarray(np.asarray(inputs["qW1"], f).T.astype(bf)),
        # fold s1 into Wq2 columns (input-channel scaling); transposed layout [in, out]
        "wq2": np.ascontiguousarray((qW2 * s1[None, :]).T.astype(f).astype(bf)),
        "wk1": np.ascontiguousarray(np.asarray(inputs["kW1"], f).T),
        "wk2": np.ascontiguousarray(np.asarray(inputs["kW2"], f).T),
        "wv": np.ascontiguousarray(np.asarray(inputs["vW"], f).T),
        # fold s_u into Wu rows (output-channel scaling); transposed layout [in, out]
        "wu": np.ascontiguousarray((uW * su[:, None]).T.astype(f)),
        "bq1": pack(b1 / s1), "bq2": pack(b2 / s2),
        "sk1": pack(sk1), "bk1": pack(bk1), "sk2": pack(sk2), "bk2": pack(bk2),
        "sq2": pack(s2), "sv": pack(sv), "bv": pack(bv), "bu": pack(bu),
    }
    x = np.asarray(inputs["x"], f)
    ctx = np.asarray(inputs["context"], f)
    in_maps = []
    for b_i in range(x.shape[0]):
        m = dict(base)
        m["x"] = np.ascontiguousarray(x[b_i].reshape(C, -1)[:, :npix].astype(bf))
        m["ctxt"] = np.ascontiguousarray(ctx[b_i].reshape(C, NCTX))
        in_maps.append(m)
    return in_maps


_NC_CACHE = {}


def _get_nc(npix=NPIX):
    key = (npix, TN)
    if key not in _NC_CACHE:
        _NC_CACHE[key] = _build(npix)
    return _NC_CACHE[key]


def run(inputs, trace=False, **kwargs):
    """Run on 8 cores; returns (y [8,512,128,128], BassKernelResults)."""
    nc = _get_nc()
    in_maps = _prepare_inputs(inputs)
    res = bass_utils.run_bass_kernel_spmd(
        nc, in_maps, core_ids=list(range(B)), trace=trace, **kwargs)
    y = np.stack([np.asarray(res.results[b]["y"], np.float32).reshape(C, H, W)
                  for b in range(B)])
    return y, res


def kernel(**inputs):
    y, _ = run(inputs)
    return y


# revision 17
# speedup vs baseline: 1.2321x; 1.2321x over previous
"""Trainium2 Bass kernel for nn_ContextRelation_Module (dense_transformer).

Data-parallel over batch: 8 batches -> 8 NeuronCores, one batch each.

Per-core program (B=1 slice), algebraically restructured vs the module:

  x     [512, 16384]   (C_in, H*W), bf16
  q1'   = relu(Wq1 @ x + bq1/s1)                     [256, T]  (s1 folded into Wq2)
  q2'   = relu(Wq2' @ q1' + bq2/s2)                  [256, T]  (s2 folded into k2)
  sim   = k2'^T @ q2'                                [19, T]
  esim  = exp(sim / 16)                 (ACT, [19,T])
  denom = partition-reduce(esim)        (Pool, [1,T])
  recip = 1/denom                       (DVE custom approx, [1,T])
  recipB= ones19 ^T recip               (PE broadcast matmul, [19,T])
  attT  = esim * recipB  -> bf16        (DVE, [19,T])
  y     = relu(uvT^T @ attT + bu)       [512, T], bf16 out

where uvT = (Wu' @ v)^T [19, 512] is computed once in the preamble --
u-projection folded through the (linear) attention-weighted sum, removing
the [256,T] ctx intermediate entirely.  All BN scales are folded into the
next layer's weights (host- or preamble-side) so every PSUM drain is a
single instruction: relu(psum + bias) on either ACT or DVE.

Big-GEMM operands (x, Wq1, Wq2', k2', uvT, attT) are bf16; accumulation is
fp32 in PSUM; the k/v/uv preamble stays fp32.  x and y move over HBM as
bf16 (y is converted back to fp32 on the host).
"""

import numpy as np

import concourse.bacc as bacc
import concourse.bass as bass
import concourse.bass_isa as bass_isa
import concourse.mybir as mybir
import concourse.tile as tile
from concourse import bass_utils
from concourse.bass import ts

AFT = mybir.ActivationFunctionType
ALU = mybir.AluOpType
F32 = mybir.dt.float32
BF16 = mybir.dt.bfloat16

# problem dims (hardcoded per contract)
B = 8
C = 512            # input/output channels
K = 256            # key_channels
H = 128
W = 128
NCTX = 19          # context tokens
NPIX = H * W       # 16384 pixels per batch
CB = C // 128      # 4 partition blocks of C
KB = K // 128      # 2 partition blocks of K
EPS = 1e-5
SOFTMAX_SCALE = K ** -0.5   # 1/16

# tunables
TN = 512                       # free-dim tile (one PSUM bank of fp32)


def _build(npix=NPIX, repeat=1):
    """Build + compile the per-core Bass module."""
    nt = npix // TN
    nc = bacc.Bacc("TRN2", target_bir_lowering=False, debug=False)

    x_d = nc.dram_tensor("x", [C, npix], BF16, kind="ExternalInput").ap()
    ct_d = nc.dram_tensor("ctxt", [C, NCTX], F32, kind="ExternalInput").ap()
    wq1_d = nc.dram_tensor("wq1", [C, K], BF16, kind="ExternalInput").ap()
    wq2_d = nc.dram_tensor("wq2", [K, K], BF16, kind="ExternalInput").ap()
    wk1_d = nc.dram_tensor("wk1", [C, K], F32, kind="ExternalInput").ap()
    wk2_d = nc.dram_tensor("wk2", [K, K], F32, kind="ExternalInput").ap()
    wv_d = nc.dram_tensor("wv", [C, K], F32, kind="ExternalInput").ap()
    wu_d = nc.dram_tensor("wu", [K, C], F32, kind="ExternalInput").ap()
    # packed per-channel vectors: [128, nblk], channel = blk*128 + p
    sb_names = ["bq1", "bq2", "sk1", "bk1", "sk2", "bk2", "sq2", "sv", "bv", "bu"]
    sb_d = {}
    for n in sb_names:
        nblk = CB if n == "bu" else KB
        sb_d[n] = nc.dram_tensor(n, [128, nblk], F32, kind="ExternalInput").ap()
    y_d = nc.dram_tensor("y", [C, npix], BF16, kind="ExternalOutput").ap()

    x_v = x_d.rearrange("(c p) n -> p c n", p=128)
    y_v = y_d.rearrange("(c p) n -> p c n", p=128)

    with tile.TileContext(nc) as tc, nc.allow_low_precision(reason="bf16 matmul operands"):
        with (
            tc.tile_pool(name="consts", bufs=1) as consts,
            tc.tile_pool(name="xin", bufs=3) as xin,
            tc.tile_pool(name="yout", bufs=3) as yout,
            tc.tile_pool(name="work", bufs=2) as work,
            tc.tile_pool(name="psA", bufs=2, space="PSUM") as psA,      # q1/q2 [128,512]
            tc.tile_pool(name="psS", bufs=2, space="PSUM") as psS,      # sim [19,512]
            tc.tile_pool(name="psQ", bufs=2, space="PSUM") as psQ,      # denom/bcast [19,512]
            tc.tile_pool(name="psY", bufs=2, space="PSUM") as psY,      # y [128,512]
        ):
            # ---- constants ----
            wq1_sb = consts.tile([128, CB, K], BF16, name="wq1_sb")
            nc.sync.dma_start(out=wq1_sb, in_=wq1_d.rearrange("(c p) m -> p c m", p=128))
            wq2_sb = consts.tile([128, KB, K], BF16, name="wq2_sb")
            nc.sync.dma_start(out=wq2_sb, in_=wq2_d.rearrange("(c p) m -> p c m", p=128))
            wk1_sb = consts.tile([128, CB, K], F32, name="wk1_sb")
            nc.sync.dma_start(out=wk1_sb, in_=wk1_d.rearrange("(c p) m -> p c m", p=128))
            wk2_sb = consts.tile([128, KB, K], F32, name="wk2_sb")
            nc.sync.dma_start(out=wk2_sb, in_=wk2_d.rearrange("(c p) m -> p c m", p=128))
            wv_sb = consts.tile([128, CB, K], F32, name="wv_sb")
            nc.sync.dma_start(out=wv_sb, in_=wv_d.rearrange("(c p) m -> p c m", p=128))
            wu_sb = consts.tile([128, KB, C], F32, name="wu_sb")
            nc.sync.dma_start(out=wu_sb, in_=wu_d.rearrange("(c p) m -> p c m", p=128))
            sb = {}
            for n in sb_names:
                nblk = CB if n == "bu" else KB
                t_ = consts.tile([128, nblk], F32, name=f"{n}_sb")
                nc.sync.dma_start(out=t_, in_=sb_d[n])
                sb[n] = t_
            ct_sb = consts.tile([128, CB, NCTX], F32, name="ct_sb")
            nc.sync.dma_start(out=ct_sb, in_=ct_d.rearrange("(c p) m -> p c m", p=128))

            ones19 = consts.tile([NCTX, 1], BF16, name="ones19")
            nc.vector.memset(ones19, 1.0)
            ones1 = consts.tile([1, NCTX], BF16, name="ones1")
            nc.vector.memset(ones1, 1.0)

            # ---- preamble: k2' (bf16, s_q2-scaled), uvT (bf16) ----
            k1_sb = consts.tile([128, KB, NCTX], F32, name="k1_sb")
            for m in range(KB):
                pf = psA.tile([128, TN], F32, tag="mm", name="pk1")
                p = pf[:, :NCTX]
                for c in range(CB):
                    nc.tensor.matmul(p, wk1_sb[:, c, ts(m, 128)], ct_sb[:, c, :],
                                     start=(c == 0), stop=(c == CB - 1))
                nc.scalar.activation(k1_sb[:, m, :], p, AFT.Relu,
                                     bias=sb["bk1"][:, m:m + 1], scale=sb["sk1"][:, m:m + 1])
            k2_sb = consts.tile([128, KB, NCTX], BF16, name="k2_sb")
            for m in range(KB):
                pf = psA.tile([128, TN], F32, tag="mm", name="pk2")
                p = pf[:, :NCTX]
                for c in range(KB):
                    nc.tensor.matmul(p, wk2_sb[:, c, ts(m, 128)], k1_sb[:, c, :],
                                     start=(c == 0), stop=(c == KB - 1))
                # relu(s*psum+b) then * s_q2 (folded from the q2 BN scale)
                kf = consts.tile([128, NCTX], F32, name=f"k2f{m}")
                nc.scalar.activation(kf, p, AFT.Relu,
                                     bias=sb["bk2"][:, m:m + 1], scale=sb["sk2"][:, m:m + 1])
                nc.vector.tensor_scalar_mul(k2_sb[:, m, :], kf, sb["sq2"][:, m:m + 1])
            v_sb = consts.tile([128, KB, NCTX], F32, name="v_sb")
            for m in range(KB):
                pf = psA.tile([128, TN], F32, tag="mm", name="pv")
                p = pf[:, :NCTX]
                for c in range(CB):
                    nc.tensor.matmul(p, wv_sb[:, c, ts(m, 128)], ct_sb[:, c, :],
                                     start=(c == 0), stop=(c == CB - 1))
                nc.scalar.activation(v_sb[:, m, :], p, AFT.Relu,
                                     bias=sb["bv"][:, m:m + 1], scale=sb["sv"][:, m:m + 1])
            # uvT [19, C] = v^T @ Wu'^T  (s_u already folded into wu host-side)
            puv = psS.tile([NCTX, TN], F32, tag="sim", name="puv")
            for c in range(KB):
                nc.tensor.matmul(puv, v_sb[:, c, :], wu_sb[:, c, :],
                                 start=(c == 0), stop=(c == KB - 1))
            uvT_sb = consts.tile([NCTX, C], BF16, name="uvT_sb")
            nc.scalar.activation(uvT_sb, puv, AFT.Copy)

            # ---- main loop, software-pipelined ----
            state = {}

            def s0a(t):  # x dma + q1 m0
                xt = xin.tile([128, CB, TN], BF16, tag="xt", name="xt")
                nc.sync.dma_start(out=xt, in_=x_v[:, :, ts(t, TN)])
                q1 = work.tile([128, KB, TN], BF16, tag="q1", name="q1")
                p = psA.tile([128, TN], F32, tag="mm", name="pq1a")
                for c in range(CB):
                    nc.tensor.matmul(p, wq1_sb[:, c, ts(0, 128)], xt[:, c, :],
                                     start=(c == 0), stop=(c == CB - 1))
                nc.scalar.activation(q1[:, 0, :], p, AFT.Relu, bias=sb["bq1"][:, 0:1])
                state[t] = {"xt": xt, "q1": q1}

            def s0b(t):  # q1 m1
                st = state[t]
                p = psA.tile([128, TN], F32, tag="mm", name="pq1b")
                for c in range(CB):
                    nc.tensor.matmul(p, wq1_sb[:, c, ts(1, 128)], st["xt"][:, c, :],
                                     start=(c == 0), stop=(c == CB - 1))
                nc.vector.tensor_scalar(st["q1"][:, 1, :], p,
                                        sb["bq1"][:, 1:2], 0.0, ALU.add, ALU.max)

            def s0c(t):  # q2 both blocks
                st = state[t]
                q2 = work.tile([128, KB, TN], BF16, tag="q2", name="q2")
                for m in range(KB):
                    p = psA.tile([128, TN], F32, tag="mm", name="pq2")
                    for c in range(KB):
                        nc.tensor.matmul(p, wq2_sb[:, c, ts(m, 128)], st["q1"][:, c, :],
                                         start=(c == 0), stop=(c == KB - 1))
                    if m == 0:
                        nc.scalar.activation(q2[:, m, :], p, AFT.Relu, bias=sb["bq2"][:, m:m + 1])
                    else:
                        nc.vector.tensor_scalar(q2[:, m, :], p,
                                                sb["bq2"][:, m:m + 1], 0.0, ALU.add, ALU.max)
                st["q2"] = q2

            def s0d(t):  # sim [19, TN]
                st = state[t]
                psim = psS.tile([NCTX, TN], F32, tag="sim", name="psim")
                for c in range(KB):
                    nc.tensor.matmul(psim, k2_sb[:, c, :], st["q2"][:, c, :],
                                     start=(c == 0), stop=(c == KB - 1))
                st["psim"] = psim

            def s1(t):  # exp
                st = state[t]
                esim = work.tile([NCTX, TN], BF16, tag="esim", name="esim", bufs=3)
                nc.scalar.activation(esim, st["psim"], AFT.Exp, scale=SOFTMAX_SCALE)
                st["esim"] = esim

            def sP(t):  # denominator [1,T] via ones-matmul + fast reciprocal
                st = state[t]
                pden = psQ.tile([NCTX, TN], F32, tag="pq", name="pden")
                nc.tensor.matmul(pden[0:1, :], ones19, st["esim"], start=True, stop=True)
                recip1 = work.tile([1, TN], F32, tag="recip1", name="recip1")
                nc.vector.reciprocal_approx_fast(out=recip1, in_=pden[0:1, :])
                # bridge to bf16 for the broadcast matmul (Pool engine is idle)
                recipb = work.tile([1, TN], BF16, tag="recipb", name="recipb")
                nc.gpsimd.tensor_copy(recipb, recip1)
                st["recipb"] = recipb

            def sQ(t):  # broadcast recip across 19 rows + normalize -> bf16
                st = state[t]
                pbc = psQ.tile([NCTX, TN], F32, tag="pq", name="pbc")
                nc.tensor.matmul(pbc, ones1, st["recipb"], start=True, stop=True)
                attT = work.tile([NCTX, TN], BF16, tag="attT", name="attT")
                nc.vector.tensor_mul(attT, st["esim"], pbc)
                st["attT"] = attT

            def s3(t, ms):  # y blocks
                st = state[t]
                if "yt" not in st:
                    st["yt"] = yout.tile([128, CB, TN], BF16, tag="yt", name="yt")
                yt = st["yt"]
                for m in ms:
                    p = psY.tile([128, TN], F32, tag="y", name="py")
                    nc.tensor.matmul(p, uvT_sb[:, ts(m, 128)], st["attT"],
                                     start=True, stop=True)
                    if m % 2 == 0:
                        nc.scalar.activation(yt[:, m, :], p, AFT.Relu, bias=sb["bu"][:, m:m + 1])
                    else:
                        nc.vector.tensor_scalar(yt[:, m, :], p,
                                                sb["bu"][:, m:m + 1], 0.0, ALU.add, ALU.max)
                if ms[-1] == CB - 1:
                    nc.gpsimd.dma_start(out=y_v[:, :, ts(t, TN)], in_=yt)
                    state.pop(t)

            for r in range(repeat):
                for t in range(nt + 6):
                    if 5 <= t <= nt + 4:
                        s3(t - 5, [0, 1])
                    if t < nt:
                        s0a(t)
                    if 1 <= t <= nt:
                        s0d(t - 1)
                    if 5 <= t <= nt + 4:
                        s3(t - 5, [2, 3])
                    if t < nt:
                        s0b(t)
                    if 2 <= t <= nt + 1:
                        s1(t - 2)
                    if 3 <= t <= nt + 2:
                        sP(t - 3)
                    if t < nt:
                        s0c(t)
                    if 4 <= t <= nt + 3:
                        sQ(t - 4)

    nc.compile()
    return nc


def _prepare_inputs(inputs, npix=NPIX):
    """Fold BN into weights/biases, transpose, shard over batch."""
    import ml_dtypes
    f = np.float32
    bf = ml_dtypes.bfloat16

    def fold(bn, conv_b):
        g, be, m, v = [np.asarray(a, dtype=np.float64) for a in bn]
        s = g / np.sqrt(v + EPS)
        t = be - m * s
        bias = np.asarray(conv_b, dtype=np.float64) * s + t
        return s, bias

    def pack(vec):  # [C'] -> [128, C'//128], channel = blk*128 + p
        return np.ascontiguousarray(np.asarray(vec, f).reshape(-1, 128).T)

    s1, b1 = fold(inputs["qbn1"], inputs["qb1"])
    s2, b2 = fold(inputs["qbn2"], inputs["qb2"])
    sk1, bk1 = fold(inputs["kbn1"], inputs["kb1"])
    sk2, bk2 = fold(inputs["kbn2"], inputs["kb2"])
    sv, bv = fold(inputs["vbn"], inputs["vb"])
    su, bu = fold(inputs["ubn"], inputs["ub"])

    qW2 = np.asarray(inputs["qW2"], np.float64)
    uW = np.asarray(inputs["uW"], np.float64)

    base = {
        "wq1": np.ascontiguousarray(np.asarray(inputs["qW1"], f).T.astype(bf)),
        # fold s1 into Wq2 columns (input-channel scaling); transposed layout [in, out]
        "wq2": np.ascontiguousarray((qW2 * s1[None, :]).T.astype(f).astype(bf)),
        "wk1": np.ascontiguousarray(np.asarray(inputs["kW1"], f).T),
        "wk2": np.ascontiguousarray(np.asarray(inputs["kW2"], f).T),
        "wv": np.ascontiguousarray(np.asarray(inputs["vW"], f).T),
        # fold s_u into Wu rows (output-channel scaling); transposed layout [in, out]
        "wu": np.ascontiguousarray((uW * su[:, None]).T.astype(f)),
        "bq1": pack(b1 / s1), "bq2": pack(b2 / s2),
        "sk1": pack(sk1), "bk1": pack(bk1), "sk2": pack(sk2), "bk2": pack(bk2),
        "sq2": pack(s2), "sv": pack(sv), "bv": pack(bv), "bu": pack(bu),
    }
    x = np.asarray(inputs["x"], f)
    ctx = np.asarray(inputs["context"], f)
    in_maps = []
    for b_i in range(x.shape[0]):
        m = dict(base)
        m["x"] = np.ascontiguousarray(x[b_i].reshape(C, -1)[:, :npix].astype(bf))
        m["ctxt"] = np.ascontiguousarray(ctx[b_i].reshape(C, NCTX))
        in_maps.append(m)
    return in_maps


_NC_CACHE = {}


def _get_nc(npix=NPIX):
    key = (npix, TN)
    if key not in _NC_CACHE:
        _NC_CACHE[key] = _build(npix)
    return _NC_CACHE[key]


def run(inputs, trace=False, **kwargs):
    """Run on 8 cores; returns (y [8,512,128,128], BassKernelResults)."""
    nc = _get_nc()
    in_maps = _prepare_inputs(inputs)
    res = bass_utils.run_bass_kernel_spmd(
        nc, in_maps, core_ids=list(range(B)), trace=trace, **kwargs)
    y = np.stack([np.asarray(res.results[b]["y"], np.float32).reshape(C, H, W)
                  for b in range(B)])
    return y, res


def kernel(**inputs):
    y, _ = run(inputs)
    return y
